# revision 15
# baseline (speedup 1.0000x reference)
"""Causal attention (QKV proj + softmax + PV + ReLU) on 8 trn2 NeuronCores.

Sharding: data-parallel over batch B=32 -> 4 batches per core; projection
weights replicated.

Dtypes: the Q/K path runs in fp8-e4m3 with DoubleRow matmuls (two 128-row
contraction slabs per pass -> 2x PE throughput vs fp16); V / P / PV stay
fp16 so V-path error never hits the output directly. Projection weights are
pre-scaled by 16 on the host so their U(-1/32,1/32) entries escape e4m3's
subnormal range; the 1/16 descale is folded into the PSUM->SBUF drain.
Accumulation is always fp32 in PSUM.

Host prep: x is packed twice: fp8 slab-major [128, CT, L] for the Q/K
projections, and fp16 [C, L] for the V projection; weights pretransposed
(fp8 ones pre-packed slab-major); biases/masks prepacked.

Per core, per batch:
  Q^T,K^T[d,l] = Wqk8.T @ X8  (4 DoubleRow passes instead of 8; descale +
      bias folded into the DVE drain, output straight to fp8 tiles)
  V[l,d] = X^T.T @ Wv^T        (fp16; bias via DVE add during drain)
  S^T[j,i] tiles = K8.T @ Q8 (2 DoubleRow passes), causal-sparse
  P^T = exp(scale*S^T + padmask_bias_j)  (one ACT op; per-partition bias
      handles the padding mask exactly; no max-subtraction needed since
      logits are provably small), diagonal tile masked by a DVE multiply
  O' = P^T.T @ V and rowsum = P^T.T @ ones share the stationary operand
  out = Relu(O' * (1/rowsum))  (one DVE tensor_scalar: mult then max-0)

DMA queues are program-ordered per engine: x prefetch on nc.sync, const
loads on nc.scalar, stores on nc.gpsimd. Dummy warmup matmuls pre-warm the
PE clock-gate while batch-0 inputs stream in.
"""

import os
from contextlib import ExitStack

import numpy as np
import ml_dtypes

import concourse.tile as tile
from concourse import bacc, mybir
from concourse import bass_utils

F32 = mybir.dt.float32
F16 = mybir.dt.float16
F8 = mybir.dt.float8e4
E4 = ml_dtypes.float8_e4m3
AF = mybir.ActivationFunctionType
DRM = mybir.MatmulPerfMode.DoubleRow

N_CORES = 8
B = 32
L = 1024
C = 1024  # d_model
D = 512
P = 128
NB = B // N_CORES  # batches per core
CT = C // P  # 8 contraction slabs
DT = D // P  # 4 d slabs
LT = L // P  # 8 l/j/i tiles
SCALE = float(D) ** -0.5
NEG = -30000.0
WSCALE = 16.0  # fp8 weight pre-scale (escapes e4m3 subnormals)


def build_program(nb: int = NB):
    """Build the per-core Bass program for nb batches."""
    nc = bacc.Bacc("TRN2", target_bir_lowering=False, debug=False,
                   num_devices=N_CORES)

    x8b = nc.dram_tensor("x8b", [nb, P, CT, L], F8, kind="ExternalInput").ap()
    xtb = nc.dram_tensor("xtb", [nb, C, L], F16, kind="ExternalInput").ap()
    wq8 = nc.dram_tensor("wq8", [P, CT, D], F8, kind="ExternalInput").ap()
    wk8 = nc.dram_tensor("wk8", [P, CT, D], F8, kind="ExternalInput").ap()
    wvT = nc.dram_tensor("wvT", [C, D], F16, kind="ExternalInput").ap()
    wqT = nc.dram_tensor("wqT", [C, D], F16, kind="ExternalInput").ap()
    wkT = nc.dram_tensor("wkT", [C, D], F16, kind="ExternalInput").ap()
    bq2 = nc.dram_tensor("bq2", [P, DT], F32, kind="ExternalInput").ap()
    bk2 = nc.dram_tensor("bk2", [P, DT], F32, kind="ExternalInput").ap()
    bvb = nc.dram_tensor("bvb", [P, D], F32, kind="ExternalInput").ap()
    bqb = nc.dram_tensor("bqb", [P, D], F32, kind="ExternalInput").ap()
    bkb = nc.dram_tensor("bkb", [P, D], F32, kind="ExternalInput").ap()
    pmt = nc.dram_tensor("pmt", [nb, P, LT], F32, kind="ExternalInput").ap()
    tri = nc.dram_tensor("tri", [P, P], F16, kind="ExternalInput").ap()
    idm = nc.dram_tensor("idm", [P, P], F16, kind="ExternalInput").ap()
    out = nc.dram_tensor("out", [nb, L, D], F32, kind="ExternalOutput").ap()

    with tile.TileContext(nc) as tc, ExitStack() as ctx:
        const = ctx.enter_context(tc.tile_pool(name="const", bufs=1))
        x8_pool = ctx.enter_context(tc.tile_pool(name="x8", bufs=2))
        xt_pool = ctx.enter_context(tc.tile_pool(name="xt", bufs=2))
        qk_pool = ctx.enter_context(tc.tile_pool(name="qk", bufs=2))
        blk_pool = ctx.enter_context(tc.tile_pool(name="blk", bufs=2))
        v_pool = ctx.enter_context(tc.tile_pool(name="v", bufs=2))
        pt_pool = ctx.enter_context(tc.tile_pool(name="pt", bufs=2))
        o_pool = ctx.enter_context(tc.tile_pool(name="o", bufs=3))
        sm_pool = ctx.enter_context(tc.tile_pool(name="sm", bufs=4))
        pm_pool = ctx.enter_context(tc.tile_pool(name="pm", bufs=2))
        mm_ps = ctx.enter_context(tc.tile_pool(name="mmps", bufs=4, space="PSUM"))
        o_ps = ctx.enter_context(tc.tile_pool(name="ops", bufs=2, space="PSUM"))
        r_ps = ctx.enter_context(tc.tile_pool(name="rps", bufs=2, space="PSUM"))

        # --- constants, loaded once; on the scalar HWDGE queue so the
        # sync queue is dedicated to x prefetch. Tiny tensors first (they
        # unblock drains early), then weights in first-use order. ---
        bq_sb = const.tile([P, DT], F32)
        nc.scalar.dma_start(bq_sb[:], bq2[:])
        bk_sb = const.tile([P, DT], F32)
        nc.scalar.dma_start(bk_sb[:], bk2[:])
        bv_sb = const.tile([P, D], F32)
        nc.scalar.dma_start(bv_sb[:], bvb[:])
        bqb_sb = const.tile([P, D], F32)
        nc.scalar.dma_start(bqb_sb[:], bqb[:])
        bkb_sb = const.tile([P, D], F32)
        nc.scalar.dma_start(bkb_sb[:], bkb[:])
        tri_sb = const.tile([P, P], F16)
        nc.scalar.dma_start(tri_sb[:], tri[:])
        idm_sb = const.tile([P, P], F16)
        nc.scalar.dma_start(idm_sb[:], idm[:])
        wq_sb = const.tile([P, CT, D], F8)
        nc.scalar.dma_start(wq_sb[:], wq8[:])
        wk_sb = const.tile([P, CT, D], F8)
        nc.scalar.dma_start(wk_sb[:], wk8[:])
        wq16_sb = const.tile([P, CT, D], F16)
        nc.scalar.dma_start(wq16_sb[:],
                            wqT.rearrange("(t p) d -> p t d", p=P))
        wk16_sb = const.tile([P, CT, D], F16)
        nc.scalar.dma_start(wk16_sb[:],
                            wkT.rearrange("(t p) d -> p t d", p=P))
        wv_sb = const.tile([P, CT, D], F16)
        ones_sb = const.tile([P, 1], F16)
        nc.vector.memset(ones_sb[:], 1.0)

        # PE warmup: dummy matmuls with no input deps keep the PE busy while
        # batch-0 inputs stream in, so the HAM clock-gate is already at
        # 2.4 GHz when the real stream starts.
        warm_sb = const.tile([P, 512], F16)
        nc.vector.memset(warm_sb[:], 0.0)
        for w in range(15):
            wps = mm_ps.tile([P, 512], F32, tag="ps", name=f"warm{w}")
            nc.tensor.matmul(wps[:], warm_sb[:, 0:P], warm_sb[:],
                             start=True, stop=True)

        for b in range(nb):
            # --- X fp8 slab-major [128, CT, L] for Q/K ---
            x8 = x8_pool.tile([P, CT, L], F8, tag="x8", name=f"x8_{b}")
            if b == 0:
                # first batch: l<512 halves first so Q lc=0 starts earlier
                nc.sync.dma_start(x8[:, :, 0:512], x8b[b][:, :, 0:512])
                nc.sync.dma_start(x8[:, :, 512:L], x8b[b][:, :, 512:L])
            else:
                nc.sync.dma_start(x8[:], x8b[b])
            # --- X^T fp16 tiles [128c, 1024l] for V + early block ---
            xt = []
            if b == 0:
                # l<128 slices first: the early-block projs (and V lt=0)
                # depend only on these; wv rides behind (V starts ~17us in)
                for ct in range(CT):
                    t = xt_pool.tile([P, L], F16, tag=f"xt{ct}",
                                     name=f"xt{ct}_{b}")
                    nc.sync.dma_start(t[:, 0:P],
                                      xtb[b, ct * P:(ct + 1) * P, 0:P])
                    xt.append(t)
                for ct in range(CT):
                    nc.sync.dma_start(xt[ct][:, P:L],
                                      xtb[b, ct * P:(ct + 1) * P, P:L])
                nc.sync.dma_start(wv_sb[:],
                                  wvT.rearrange("(t p) d -> p t d", p=P))
            else:
                for ct in range(CT):
                    t = xt_pool.tile([P, L], F16, tag=f"xt{ct}",
                                     name=f"xt{ct}_{b}")
                    nc.sync.dma_start(t[:], xtb[b, ct * P:(ct + 1) * P, :])
                    xt.append(t)
            pm_sb = pm_pool.tile([P, LT], F32, name=f"pm_{b}")
            nc.sync.dma_start(pm_sb[:], pmt[b])

            # --- Q^T, K^T: fp8 [128, DT, L] tiles; DoubleRow over 2-slab
            # pairs; descale+bias folded into the DVE drain ---
            qt = qk_pool.tile([P, DT, L], F8, tag="qt", name=f"qt_{b}")
            kt = qk_pool.tile([P, DT, L], F8, tag="kt", name=f"kt_{b}")
            for name, w_sb, b_sb, dst in (("q", wq_sb, bq_sb, qt),
                                          ("k", wk_sb, bk_sb, kt)):
                if b == 0 and name == "q":
                    # lc-outer so all lc=0 groups run on the early halves
                    order = [(dt, lc) for lc in range(L // 512)
                             for dt in range(DT)]
                else:
                    order = [(dt, lc) for dt in range(DT)
                             for lc in range(L // 512)]
                for dt, lc in order:
                    ps = mm_ps.tile([P, 512], F32, tag="ps",
                                      name=f"{name}ps{dt}_{lc}_{b}")
                    for s in range(CT // 2):
                        nc.tensor.matmul(
                            ps[:],
                            w_sb[:, 2 * s:2 * s + 2, dt * P:(dt + 1) * P],
                            x8[:, 2 * s:2 * s + 2, lc * 512:(lc + 1) * 512],
                            start=(s == 0), stop=(s == CT // 2 - 1),
                            perf_mode=DRM)
                    nc.vector.tensor_scalar(
                        dst[:, dt, lc * 512:(lc + 1) * 512], ps[:],
                        1.0 / WSCALE, b_sb[:, dt:dt + 1],
                        mybir.AluOpType.mult, mybir.AluOpType.add)

            # --- early-block q,k [l<128, d] fp16 (V-style x-stationary
            # matmuls, full N=512): rows i<128 of the attention need an
            # accurate fp16 path — fp8 logit noise doesn't average out in
            # their few-term softmaxes ---
            blkq = blk_pool.tile([P, D], F16, tag="blkq", name=f"blkq_{b}")
            blkk = blk_pool.tile([P, D], F16, tag="blkk", name=f"blkk_{b}")
            for w16_sb, bb_sb, blk in ((wq16_sb, bqb_sb, blkq),
                                       (wk16_sb, bkb_sb, blkk)):
                ps = mm_ps.tile([P, D], F32, tag="ps", name=f"blkps_{b}")
                for ct in range(CT):
                    nc.tensor.matmul(ps[:], xt[ct][:, 0:P], w16_sb[:, ct, :],
                                     start=(ct == 0), stop=(ct == CT - 1))
                nc.vector.tensor_add(blk[:], ps[:], bb_sb[:])

            # --- V: [128l, 512d] fp16 tiles; bias via DVE add at drain;
            # the 8 early-block PE transposes (-> d-major qt16/kt16) are
            # interleaved between V groups so their LDWEIGHTS hide under
            # V's N=512 matmuls ---
            qt16 = blk_pool.tile([P, DT, P], F16, tag="qt16", name=f"qt16_{b}")
            kt16 = blk_pool.tile([P, DT, P], F16, tag="kt16", name=f"kt16_{b}")
            tponder = [(blkq, qt16, ds) for ds in range(DT)] + \
                      [(blkk, kt16, ds) for ds in range(DT)]
            v = []
            for lt in range(LT):
                t = v_pool.tile([P, D], F16, tag=f"v{lt}", name=f"v{lt}_{b}")
                v.append(t)
                ps = mm_ps.tile([P, D], F32, tag="ps", name=f"vps{lt}_{b}")
                for ct in range(CT):
                    nc.tensor.matmul(ps[:], xt[ct][:, lt * P:(lt + 1) * P],
                                     wv_sb[:, ct, :],
                                     start=(ct == 0), stop=(ct == CT - 1))
                nc.vector.tensor_add(t[:], ps[:], bv_sb[:])
                if lt < len(tponder):
                    blk, dstT, ds = tponder[lt]
                    tps = mm_ps.tile([P, P], F16, tag="ps",
                                     name=f"tps{lt}_{b}")
                    nc.tensor.transpose(tps[:], blk[:, ds * P:(ds + 1) * P],
                                        idm_sb[:])
                    nc.vector.tensor_copy(dstT[:, ds, :], tps[:])

            # --- S^T tiles + exp -> P^T (causal: only i >= j0 computed),
            # interleaved with the PV phase (stagger 2): PV(ib) matmuls give
            # the PE work while ACT drains S(jb) chunks, so the 2x-faster
            # fp8 S matmuls never stall on exp, and the last batch's exp
            # latency is not exposed at the end of the program ---
            pt = []

            def emit_s(jb):
                j0 = jb * P
                t = pt_pool.tile([P, L], F16, tag=f"pt{jb}", name=f"pt{jb}_{b}")
                pt.append(t)
                i0 = j0
                if jb == 0:
                    # rows i<128: accurate fp16 path from qt16/kt16
                    ps = mm_ps.tile([P, P], F32, tag="ps", name=f"sps0h_{b}")
                    for dt in range(DT):
                        nc.tensor.matmul(ps[:], kt16[:, dt, :], qt16[:, dt, :],
                                         start=(dt == 0), stop=(dt == DT - 1))
                    nc.scalar.activation(t[:, 0:P], ps[:], AF.Exp,
                                         bias=pm_sb[:, 0:1], scale=SCALE)
                    i0 = P
                while i0 < L:
                    n = min((i0 // 512 + 1) * 512, L) - i0
                    ps = mm_ps.tile([P, n], F32, tag="ps",
                                    name=f"sps{jb}_{i0}_{b}")
                    for s in range(DT // 2):
                        nc.tensor.matmul(
                            ps[:],
                            kt[:, 2 * s:2 * s + 2, j0:j0 + P],
                            qt[:, 2 * s:2 * s + 2, i0:i0 + n],
                            start=(s == 0), stop=(s == DT // 2 - 1),
                            perf_mode=DRM)
                    nc.scalar.activation(t[:, i0:i0 + n], ps[:], AF.Exp,
                                         bias=pm_sb[:, jb:jb + 1], scale=SCALE)
                    i0 += n
                # mask the diagonal tile: keep j<=i (upper-right triangle)
                nc.vector.tensor_mul(t[:, j0:j0 + P], t[:, j0:j0 + P],
                                     tri_sb[:])

            def emit_pv(ib):
                i0 = ib * P
                ops = o_ps.tile([P, D], F32, tag="op", name=f"ops{ib}_{b}")
                rps = r_ps.tile([P, 1], F32, tag="rp", name=f"rps{ib}_{b}")
                for jb in range(ib + 1):
                    pT = pt[jb][:, i0:i0 + P]
                    nc.tensor.matmul(ops[:], pT, v[jb][:],
                                     start=(jb == 0), stop=(jb == ib))
                    nc.tensor.matmul(rps[:], pT, ones_sb[:],
                                     start=(jb == 0), stop=(jb == ib))
                rec = sm_pool.tile([P, 1], F32, tag="rec", name=f"rec{ib}_{b}")
                nc.vector.reciprocal(rec[:], rps[:])
                o_sb = o_pool.tile([P, D], F32, tag="ot", name=f"o{ib}_{b}")
                # relu(O'/rowsum) on DVE: (in0 * rec) max 0 — keeps ACT free
                # for the exps, whose latency gates S-phase PSUM slot reuse
                nc.vector.tensor_scalar(o_sb[:], ops[:], rec[:], 0.0,
                                        mybir.AluOpType.mult,
                                        mybir.AluOpType.max)
                # SWDGE so stores never head-of-line-block the x prefetch;
                # last batch has no prefetch left, so use the faster HWDGE
                if b == nb - 1:
                    nc.sync.dma_start(out[b, i0:i0 + P, :], o_sb[:])
                else:
                    nc.gpsimd.dma_start(out[b, i0:i0 + P, :], o_sb[:])

            STAG = 2
            for jb in range(LT):
                emit_s(jb)
                if jb >= STAG:
                    emit_pv(jb - STAG)
            for ib in range(LT - STAG, LT):
                emit_pv(ib)

    nc.compile()
    return nc


def _prep_host(x, Wq, bq, Wk, bk, Wv, bv, mask):
    bf = np.float16
    # x fp16 transposed to [B, C, L] for the V projection
    xT = np.ascontiguousarray(x.transpose(0, 2, 1))
    xb16 = xT.astype(bf)
    # x fp8 slab-major [B, 128, CT, L] for Q/K DoubleRow
    x8b = np.ascontiguousarray(
        xT.reshape(B, CT, P, L).transpose(0, 2, 1, 3)).astype(E4)
    # weights: fp8 slab-major [128, CT, D], pre-scaled by WSCALE
    wq8 = np.ascontiguousarray(
        (Wq.T * WSCALE).reshape(CT, P, D).transpose(1, 0, 2)).astype(E4)
    wk8 = np.ascontiguousarray(
        (Wk.T * WSCALE).reshape(CT, P, D).transpose(1, 0, 2)).astype(E4)
    wvT = np.ascontiguousarray(Wv.T).astype(bf)
    wqT = np.ascontiguousarray(Wq.T).astype(bf)
    wkT = np.ascontiguousarray(Wk.T).astype(bf)
    bq2 = np.ascontiguousarray(
        bq.astype(np.float32).reshape(DT, P).T)  # [P, DT]
    bk2 = np.ascontiguousarray(bk.astype(np.float32).reshape(DT, P).T)
    bvb = np.ascontiguousarray(
        np.broadcast_to(bv.astype(np.float32), (P, D)))  # [P, D]
    bqb = np.ascontiguousarray(
        np.broadcast_to(bq.astype(np.float32), (P, D)))
    bkb = np.ascontiguousarray(
        np.broadcast_to(bk.astype(np.float32), (P, D)))
    pm = np.where(mask[:, 0, :] != 0, 0.0, NEG).astype(np.float32)  # [B, L]
    pmt = np.ascontiguousarray(
        pm.reshape(B, LT, P).transpose(0, 2, 1))  # [B, P, LT]
    tri = (np.arange(P)[:, None] <= np.arange(P)[None, :]).astype(bf)
    idm = np.eye(P, dtype=bf)
    return (x8b, xb16, wq8, wk8, wvT, wqT, wkT, bq2, bk2, bvb, bqb, bkb,
            pmt, tri, idm)


_NC_CACHE = {}


def kernel(x, Wq, bq, Wk, bk, Wv, bv, mask):
    x = np.asarray(x)
    Wq, bq = np.asarray(Wq), np.asarray(bq)
    Wk, bk = np.asarray(Wk), np.asarray(bk)
    Wv, bv = np.asarray(Wv), np.asarray(bv)
    mask = np.asarray(mask)

    (x8b, xb16, wq8, wk8, wvT, wqT, wkT, bq2, bk2, bvb, bqb, bkb,
     pmt, tri, idm) = _prep_host(x, Wq, bq, Wk, bk, Wv, bv, mask)

    if "nc" not in _NC_CACHE:
        _NC_CACHE["nc"] = build_program(NB)
    nc = _NC_CACHE["nc"]

    in_maps = []
    for c in range(N_CORES):
        s = slice(c * NB, (c + 1) * NB)
        in_maps.append({
            "x8b": np.ascontiguousarray(x8b[s]),
            "xtb": np.ascontiguousarray(xb16[s]),
            "wq8": wq8, "wk8": wk8, "wvT": wvT,
            "wqT": wqT, "wkT": wkT,
            "bq2": bq2, "bk2": bk2, "bvb": bvb,
            "bqb": bqb, "bkb": bkb,
            "pmt": np.ascontiguousarray(pmt[s]),
            "tri": tri, "idm": idm,
        })

    res = bass_utils.run_bass_kernel_spmd(
        nc, in_maps, core_ids=list(range(N_CORES)),
        trace=bool(int(os.environ.get("KERNEL_TRACE", "0"))),
    )
    if os.environ.get("KERNEL_RESULT_HOOK"):
        _NC_CACHE["last_result"] = res

    return np.concatenate([res.results[c]["out"] for c in range(N_CORES)],
                          axis=0)


# revision 16
# speedup vs baseline: 1.2272x; 1.2272x over previous
"""Causal attention (QKV proj + softmax + PV + ReLU) on 8 trn2 NeuronCores.

Sharding: data-parallel over batch B=32 -> 4 batches per core; projection
weights replicated.

Dtypes: everything that can tolerate it runs in fp8-e4m3 with DoubleRow
matmuls (two 128-row contraction slabs per pass -> 2x PE throughput vs
fp16): the Q/K projections, S=K^T.T@Q^T, the V projection, and P@V.
Accumulation is always fp32 in PSUM. Error control:
  - weights are pre-scaled by 16 on the host so their U(-1/32,1/32)
    entries escape e4m3's subnormal range (descale folded into drains; for
    V the 16x rides the value domain and cancels exactly in the rowsum
    normalization, via 16.0-valued `ones`).
  - rows i<128 get a full fp16 path (fp16 projections of q,k,v for l<128,
    fp16 S block, fp16 PV for ib=0): few-term softmax rows pass quant
    noise straight to the output, everything later averages it away.
    Measured rel-err 6.7e-3 vs the 2e-2 gate.
  - the fp16 early-block q,k come from V-style x-stationary N=512 matmuls
    (full PE rate) + 8 PE transposes to d-major, interleaved into the V
    phase so transpose LDWEIGHTS hide under N=512 streams.

Schedule: S(jb) and PV(ib) are interleaved (stagger 2) so PV matmuls keep
the PE busy while ACT drains exps; small constants ride in two packed
DMAs so the weight loads aren't stuck behind seven tiny descriptors; pm
biases for all batches load once. DMA queues are program-ordered per
engine: x prefetch on nc.sync, consts/weights on nc.scalar, stores on
nc.gpsimd. Dummy warmup matmuls pre-warm the PE clock-gate while batch-0
inputs stream in.
"""

import os
from contextlib import ExitStack

import numpy as np
import ml_dtypes

import concourse.tile as tile
from concourse import bacc, mybir
from concourse import bass_utils

F32 = mybir.dt.float32
F16 = mybir.dt.float16
F8 = mybir.dt.float8e4
E4 = ml_dtypes.float8_e4m3
AF = mybir.ActivationFunctionType
DRM = mybir.MatmulPerfMode.DoubleRow

N_CORES = 8
B = 32
L = 1024
C = 1024  # d_model
D = 512
P = 128
NB = B // N_CORES  # batches per core
CT = C // P  # 8 contraction slabs
DT = D // P  # 4 d slabs
LT = L // P  # 8 l/j/i tiles
SCALE = float(D) ** -0.5
NEG = -30000.0
WSCALE = 16.0  # fp8 weight pre-scale (escapes e4m3 subnormals)

# packed fp32 const layout (columns)
_BQ2, _BK2 = 0, DT
_BQB, _BKB, _BVB = 2 * DT, 2 * DT + D, 2 * DT + 2 * D
F32PACK = 2 * DT + 3 * D


def build_program(nb: int = NB):
    """Build the per-core Bass program for nb batches."""
    nc = bacc.Bacc("TRN2", target_bir_lowering=False, debug=False,
                   num_devices=N_CORES)

    x8b = nc.dram_tensor("x8b", [nb, P, CT, L], F8, kind="ExternalInput").ap()
    xtb = nc.dram_tensor("xtb", [nb, P, CT, P], F16, kind="ExternalInput").ap()
    wq8 = nc.dram_tensor("wq8", [P, CT, D], F8, kind="ExternalInput").ap()
    wk8 = nc.dram_tensor("wk8", [P, CT, D], F8, kind="ExternalInput").ap()
    wv8 = nc.dram_tensor("wv8", [P, CT, D], F8, kind="ExternalInput").ap()
    wqT = nc.dram_tensor("wqT", [C, D], F16, kind="ExternalInput").ap()
    wkT = nc.dram_tensor("wkT", [C, D], F16, kind="ExternalInput").ap()
    wvT = nc.dram_tensor("wvT", [C, D], F16, kind="ExternalInput").ap()
    cf32 = nc.dram_tensor("cf32", [P, F32PACK], F32, kind="ExternalInput").ap()
    cf16 = nc.dram_tensor("cf16", [P, 2 * P], F16, kind="ExternalInput").ap()
    pmt = nc.dram_tensor("pmt", [P, nb, LT], F32, kind="ExternalInput").ap()
    out = nc.dram_tensor("out", [nb, L, D], F32, kind="ExternalOutput").ap()

    with tile.TileContext(nc) as tc, ExitStack() as ctx:
        const = ctx.enter_context(tc.tile_pool(name="const", bufs=1))
        x8_pool = ctx.enter_context(tc.tile_pool(name="x8", bufs=2))
        xt_pool = ctx.enter_context(tc.tile_pool(name="xt", bufs=2))
        qk_pool = ctx.enter_context(tc.tile_pool(name="qk", bufs=2))
        blk_pool = ctx.enter_context(tc.tile_pool(name="blk", bufs=2))
        v_pool = ctx.enter_context(tc.tile_pool(name="v", bufs=2))
        pt_pool = ctx.enter_context(tc.tile_pool(name="pt", bufs=2))
        o_pool = ctx.enter_context(tc.tile_pool(name="o", bufs=3))
        sm_pool = ctx.enter_context(tc.tile_pool(name="sm", bufs=4))
        mm_ps = ctx.enter_context(tc.tile_pool(name="mmps", bufs=4, space="PSUM"))
        o_ps = ctx.enter_context(tc.tile_pool(name="ops", bufs=2, space="PSUM"))
        r_ps = ctx.enter_context(tc.tile_pool(name="rps", bufs=2, space="PSUM"))

        # --- constants on the scalar HWDGE queue (sync is for x prefetch).
        # wq8 first (it gates the very first matmul), tiny packs next. ---
        wq_sb = const.tile([P, CT, D], F8)
        nc.scalar.dma_start(wq_sb[:], wq8[:])
        pm_all = const.tile([P, nb, LT], F32)
        nc.scalar.dma_start(pm_all[:], pmt[:])
        cf32_sb = const.tile([P, F32PACK], F32)
        nc.scalar.dma_start(cf32_sb[:], cf32[:])
        cf16_sb = const.tile([P, 2 * P], F16)
        nc.scalar.dma_start(cf16_sb[:], cf16[:])
        wk_sb = const.tile([P, CT, D], F8)
        nc.scalar.dma_start(wk_sb[:], wk8[:])
        wq16_sb = const.tile([P, CT, D], F16)
        nc.scalar.dma_start(wq16_sb[:],
                            wqT.rearrange("(t p) d -> p t d", p=P))
        wk16_sb = const.tile([P, CT, D], F16)
        nc.scalar.dma_start(wk16_sb[:],
                            wkT.rearrange("(t p) d -> p t d", p=P))
        wv8_sb = const.tile([P, CT, D], F8)
        nc.scalar.dma_start(wv8_sb[:], wv8[:])
        wv16_sb = const.tile([P, CT, D], F16)
        nc.scalar.dma_start(wv16_sb[:],
                            wvT.rearrange("(t p) d -> p t d", p=P))

        bq_sb = cf32_sb[:, _BQ2:_BQ2 + DT]
        bk_sb = cf32_sb[:, _BK2:_BK2 + DT]
        bqb_sb = cf32_sb[:, _BQB:_BQB + D]
        bkb_sb = cf32_sb[:, _BKB:_BKB + D]
        bvb_sb = cf32_sb[:, _BVB:_BVB + D]  # 16*bv broadcast
        tri_sb = cf16_sb[:, 0:P]
        idm_sb = cf16_sb[:, P:2 * P]

        ones16_sb = const.tile([P, 1], F16)
        nc.vector.memset(ones16_sb[:], WSCALE)
        ones8_sb = const.tile([P, 2, 16], F8)
        nc.vector.memset(ones8_sb[:], WSCALE)

        # PE warmup: dummy matmuls with no input deps keep the PE busy while
        # wq8/x8 stream in, so the HAM clock-gate is at 2.4 GHz when the
        # real stream starts.
        warm_sb = const.tile([P, 512], F16)
        nc.vector.memset(warm_sb[:], 0.0)
        for w in range(9):
            wps = mm_ps.tile([P, 512], F32, tag="ps", name=f"warm{w}")
            nc.tensor.matmul(wps[:], warm_sb[:, 0:P], warm_sb[:],
                             start=True, stop=True)

        for b in range(nb):
            # --- X fp8 slab-major [128, CT, L] for Q/K/V ---
            x8 = x8_pool.tile([P, CT, L], F8, tag="x8", name=f"x8_{b}")
            if b == 0:
                # first batch: l<512 halves first so Q lc=0 starts earlier
                nc.sync.dma_start(x8[:, :, 0:512], x8b[b][:, :, 0:512])
                nc.sync.dma_start(x8[:, :, 512:L], x8b[b][:, :, 512:L])
            else:
                nc.sync.dma_start(x8[:], x8b[b])
            # --- X^T fp16 l<128 slab-major tile for the early block ---
            xt16 = xt_pool.tile([P, CT, P], F16, tag="xt", name=f"xt_{b}")
            nc.sync.dma_start(xt16[:], xtb[b])

            # --- Q^T, K^T: fp8 [128, DT, L] tiles; DoubleRow over 2-slab
            # pairs; descale+bias folded into the DVE drain ---
            qt = qk_pool.tile([P, DT, L], F8, tag="qt", name=f"qt_{b}")
            kt = qk_pool.tile([P, DT, L], F8, tag="kt", name=f"kt_{b}")
            for name, w_sb, b_sb, dst in (("q", wq_sb, bq_sb, qt),
                                          ("k", wk_sb, bk_sb, kt)):
                if b == 0 and name == "q":
                    # lc-outer so all lc=0 groups run on the early halves
                    order = [(dt, lc) for lc in range(L // 512)
                             for dt in range(DT)]
                else:
                    order = [(dt, lc) for dt in range(DT)
                             for lc in range(L // 512)]
                for dt, lc in order:
                    ps = mm_ps.tile([P, 512], F32, tag="ps",
                                    name=f"{name}ps{dt}_{lc}_{b}")
                    for s in range(CT // 2):
                        nc.tensor.matmul(
                            ps[:],
                            w_sb[:, 2 * s:2 * s + 2, dt * P:(dt + 1) * P],
                            x8[:, 2 * s:2 * s + 2, lc * 512:(lc + 1) * 512],
                            start=(s == 0), stop=(s == CT // 2 - 1),
                            perf_mode=DRM)
                    nc.vector.tensor_scalar(
                        dst[:, dt, lc * 512:(lc + 1) * 512], ps[:],
                        1.0 / WSCALE, b_sb[:, dt:dt + 1],
                        mybir.AluOpType.mult, mybir.AluOpType.add)

            # --- early-block q,k [l<128, d] fp16 (x-stationary, N=512):
            # rows i<128 need an accurate fp16 path — fp8 logit noise
            # doesn't average out in their few-term softmaxes ---
            blkq = blk_pool.tile([P, D], F16, tag="blkq", name=f"blkq_{b}")
            blkk = blk_pool.tile([P, D], F16, tag="blkk", name=f"blkk_{b}")
            for w16_sb, bb_sb, blk in ((wq16_sb, bqb_sb, blkq),
                                       (wk16_sb, bkb_sb, blkk)):
                ps = mm_ps.tile([P, D], F32, tag="ps", name=f"blkps_{b}")
                for ct in range(CT):
                    nc.tensor.matmul(ps[:], xt16[:, ct, :], w16_sb[:, ct, :],
                                     start=(ct == 0), stop=(ct == CT - 1))
                nc.vector.tensor_add(blk[:], ps[:], bb_sb)

            # --- V: fp8 pair tiles [128l, 2, 512d] (DoubleRow-ready) in the
            # 16x value domain; v0 also as fp16 for the ib=0 rim. The 8
            # early-block PE transposes (-> d-major qt16/kt16) interleave
            # so their LDWEIGHTS hide under N=512 streams ---
            qt16 = blk_pool.tile([P, DT, P], F16, tag="qt16", name=f"qt16_{b}")
            kt16 = blk_pool.tile([P, DT, P], F16, tag="kt16", name=f"kt16_{b}")
            tponder = [(blkq, qt16, ds) for ds in range(DT)] + \
                      [(blkk, kt16, ds) for ds in range(DT)]
            v8p = [v_pool.tile([P, 2, D], F8, tag=f"v8p{t}", name=f"v8p{t}_{b}")
                   for t in range(LT // 2)]
            for lt in range(LT):
                ps = mm_ps.tile([P, D], F32, tag="ps", name=f"vps{lt}_{b}")
                for s in range(CT // 2):
                    nc.tensor.matmul(
                        ps[:],
                        x8[:, 2 * s:2 * s + 2, lt * P:(lt + 1) * P],
                        wv8_sb[:, 2 * s:2 * s + 2, :],
                        start=(s == 0), stop=(s == CT // 2 - 1),
                        perf_mode=DRM)
                nc.vector.tensor_add(v8p[lt // 2][:, lt % 2, :], ps[:],
                                     bvb_sb)
                if lt < len(tponder):
                    blk, dstT, ds = tponder[lt]
                    tps = mm_ps.tile([P, P], F16, tag="ps",
                                     name=f"tps{lt}_{b}")
                    nc.tensor.transpose(tps[:], blk[:, ds * P:(ds + 1) * P],
                                        idm_sb)
                    nc.vector.tensor_copy(dstT[:, ds, :], tps[:])
            v016 = v_pool.tile([P, D], F16, tag="v016", name=f"v016_{b}")
            ps = mm_ps.tile([P, D], F32, tag="ps", name=f"v016ps_{b}")
            for ct in range(CT):
                nc.tensor.matmul(ps[:], xt16[:, ct, :], wv16_sb[:, ct, :],
                                 start=(ct == 0), stop=(ct == CT - 1))
            nc.vector.tensor_add(v016[:], ps[:], bvb_sb)

            # --- S^T tiles + exp -> P^T fp8 pair tiles (causal: only
            # i >= j0 computed), interleaved with PV (stagger 2) so PV
            # matmuls keep the PE busy while ACT drains exps ---
            ptp = [pt_pool.tile([P, 2, L], F8, tag=f"ptp{t}",
                                name=f"ptp{t}_{b}")
                   for t in range(LT // 2)]
            pt0h = pt_pool.tile([P, P], F16, tag="pt0h", name=f"pt0h_{b}")

            def emit_s(jb):
                j0 = jb * P
                tIdx, slab = jb // 2, jb % 2
                i0 = j0
                if jb == 0:
                    # rows i<128: accurate fp16 path from qt16/kt16
                    ps = mm_ps.tile([P, P], F32, tag="ps", name=f"sps0h_{b}")
                    for dt in range(DT):
                        nc.tensor.matmul(ps[:], kt16[:, dt, :], qt16[:, dt, :],
                                         start=(dt == 0), stop=(dt == DT - 1))
                    nc.scalar.activation(pt0h[:], ps[:], AF.Exp,
                                         bias=pm_all[:, b, 0:1], scale=SCALE)
                    nc.vector.tensor_mul(pt0h[:], pt0h[:], tri_sb)
                    i0 = P
                while i0 < L:
                    n = min((i0 // 512 + 1) * 512, L) - i0
                    ps = mm_ps.tile([P, n], F32, tag="ps",
                                    name=f"sps{jb}_{i0}_{b}")
                    for s in range(DT // 2):
                        nc.tensor.matmul(
                            ps[:],
                            kt[:, 2 * s:2 * s + 2, j0:j0 + P],
                            qt[:, 2 * s:2 * s + 2, i0:i0 + n],
                            start=(s == 0), stop=(s == DT // 2 - 1),
                            perf_mode=DRM)
                    nc.scalar.activation(ptp[tIdx][:, slab, i0:i0 + n], ps[:],
                                         AF.Exp, bias=pm_all[:, b, jb:jb + 1],
                                         scale=SCALE)
                    i0 += n
                if jb > 0:
                    # mask the diagonal tile: keep j<=i
                    nc.vector.tensor_mul(ptp[tIdx][:, slab, j0:j0 + P],
                                         ptp[tIdx][:, slab, j0:j0 + P],
                                         tri_sb)

            def emit_pv(ib):
                i0 = ib * P
                ops = o_ps.tile([P, D], F32, tag="op", name=f"ops{ib}_{b}")
                rps = r_ps.tile([P, 1], F32, tag="rp", name=f"rps{ib}_{b}")
                if ib == 0:
                    nc.tensor.matmul(ops[:], pt0h[:], v016[:],
                                     start=True, stop=True)
                    nc.tensor.matmul(rps[:], pt0h[:], ones16_sb[:],
                                     start=True, stop=True)
                else:
                    # fp8 DoubleRow over jb pairs; odd tail as plain fp8.
                    # rowsum piggybacks with 16.0-ones: out = (sum p*16v)
                    # / (16*sum p) — the 16x V domain cancels exactly.
                    npair = (ib + 1) // 2
                    leftover = (ib + 1) % 2
                    last = npair + leftover - 1
                    for t in range(npair):
                        st, sp = (t == 0), (t == last and not leftover)
                        pT = ptp[t][:, :, i0:i0 + P]
                        nc.tensor.matmul(ops[:], pT, v8p[t][:],
                                         start=st, stop=sp, perf_mode=DRM)
                        nc.tensor.matmul(rps[:], pT, ones8_sb[:, :, 0:1],
                                         start=st, stop=sp, perf_mode=DRM)
                    if leftover:
                        t = npair
                        pT = ptp[t][:, 0, i0:i0 + P]
                        nc.tensor.matmul(ops[:], pT, v8p[t][:, 0, :],
                                         start=(npair == 0), stop=True)
                        nc.tensor.matmul(rps[:], pT, ones8_sb[:, 0, 0:1],
                                         start=(npair == 0), stop=True)
                rec = sm_pool.tile([P, 1], F32, tag="rec", name=f"rec{ib}_{b}")
                nc.vector.reciprocal(rec[:], rps[:])
                o_sb = o_pool.tile([P, D], F32, tag="ot", name=f"o{ib}_{b}")
                # relu(O'/rowsum) on DVE: (in0 * rec) max 0 — keeps ACT free
                # for the exps, whose latency gates S-phase PSUM slot reuse
                nc.vector.tensor_scalar(o_sb[:], ops[:], rec[:], 0.0,
                                        mybir.AluOpType.mult,
                                        mybir.AluOpType.max)
                # SWDGE so stores never head-of-line-block the x prefetch;
                # last batch has no prefetch left, so use the faster HWDGE
                if b == nb - 1:
                    nc.sync.dma_start(out[b, i0:i0 + P, :], o_sb[:])
                else:
                    nc.gpsimd.dma_start(out[b, i0:i0 + P, :], o_sb[:])

            STAG = 2
            for jb in range(LT):
                emit_s(jb)
                if jb >= STAG:
                    emit_pv(jb - STAG)
            for ib in range(LT - STAG, LT):
                emit_pv(ib)

    nc.compile()
    return nc


def _prep_host(x, Wq, bq, Wk, bk, Wv, bv, mask):
    bf = np.float16
    f32 = np.float32
    xT = np.ascontiguousarray(x.transpose(0, 2, 1))  # [B, C, L]
    xs = xT.reshape(B, CT, P, L)
    # x fp8 slab-major [B, 128, CT, L] for the DoubleRow paths
    x8b = np.ascontiguousarray(xs.transpose(0, 2, 1, 3)).astype(E4)
    # x fp16 l<128 slab-major [B, 128, CT, 128] for the early block
    xtb = np.ascontiguousarray(
        xs[:, :, :, 0:P].transpose(0, 2, 1, 3)).astype(bf)

    def pack8(W):
        return np.ascontiguousarray(
            (W.T * WSCALE).reshape(CT, P, D).transpose(1, 0, 2)).astype(E4)

    wq8, wk8, wv8 = pack8(Wq), pack8(Wk), pack8(Wv)
    wqT = np.ascontiguousarray(Wq.T).astype(bf)
    wkT = np.ascontiguousarray(Wk.T).astype(bf)
    wvT = np.ascontiguousarray(Wv.T * WSCALE).astype(bf)  # 16x domain

    cf32 = np.zeros((P, F32PACK), dtype=f32)
    cf32[:, _BQ2:_BQ2 + DT] = bq.astype(f32).reshape(DT, P).T
    cf32[:, _BK2:_BK2 + DT] = bk.astype(f32).reshape(DT, P).T
    cf32[:, _BQB:_BQB + D] = bq.astype(f32)[None, :]
    cf32[:, _BKB:_BKB + D] = bk.astype(f32)[None, :]
    cf32[:, _BVB:_BVB + D] = bv.astype(f32)[None, :] * WSCALE
    cf16 = np.zeros((P, 2 * P), dtype=bf)
    cf16[:, 0:P] = (np.arange(P)[:, None] <= np.arange(P)[None, :])
    cf16[:, P:2 * P] = np.eye(P)

    pm = np.where(mask[:, 0, :] != 0, 0.0, NEG).astype(f32)  # [B, L]
    pmt = np.ascontiguousarray(
        pm.reshape(B, LT, P).transpose(2, 0, 1))  # [P, B, LT]
    return x8b, xtb, wq8, wk8, wv8, wqT, wkT, wvT, cf32, cf16, pmt


_NC_CACHE = {}


def kernel(x, Wq, bq, Wk, bk, Wv, bv, mask):
    x = np.asarray(x)
    Wq, bq = np.asarray(Wq), np.asarray(bq)
    Wk, bk = np.asarray(Wk), np.asarray(bk)
    Wv, bv = np.asarray(Wv), np.asarray(bv)
    mask = np.asarray(mask)

    (x8b, xtb, wq8, wk8, wv8, wqT, wkT, wvT, cf32, cf16, pmt) = _prep_host(
        x, Wq, bq, Wk, bk, Wv, bv, mask)

    if "nc" not in _NC_CACHE:
        _NC_CACHE["nc"] = build_program(NB)
    nc = _NC_CACHE["nc"]

    in_maps = []
    for c in range(N_CORES):
        s = slice(c * NB, (c + 1) * NB)
        in_maps.append({
            "x8b": np.ascontiguousarray(x8b[s]),
            "xtb": np.ascontiguousarray(xtb[s]),
            "wq8": wq8, "wk8": wk8, "wv8": wv8,
            "wqT": wqT, "wkT": wkT, "wvT": wvT,
            "cf32": cf32, "cf16": cf16,
            "pmt": np.ascontiguousarray(pmt[:, s]),
        })

    res = bass_utils.run_bass_kernel_spmd(
        nc, in_maps, core_ids=list(range(N_CORES)),
        trace=bool(int(os.environ.get("KERNEL_TRACE", "0"))),
    )
    if os.environ.get("KERNEL_RESULT_HOOK"):
        _NC_CACHE["last_result"] = res

    return np.concatenate([res.results[c]["out"] for c in range(N_CORES)],
                          axis=0)
